# revision 55
# baseline (speedup 1.0000x reference)
"""Distributed multi-head self-attention for Trainium2 (8 NeuronCores).

Problem: b=4, n=2048, dim=1024, heads=16, dim_head=64.
  q = x@Wq; k,v = split(x@Wkv, 2); out = softmax(q k^T / 8) v; y = out@Wout + bout

Sharding: core c <-> (batch b=c//2, head-group g=c%2). Each core computes
q/k/v + attention for its batch's 8 heads (tensor-parallel columns of
Wq/Wkv). The pair (b,0)/(b,1) AllGathers the transposed bf16 attention
outputs (per head-pair, overlapped with attention compute; the last pair
streams per-i-chunk), then each core runs the output projection with the
full Wout over ITS HALF of the sequence (selected from the gathered buffer
with per-core one-hot mask inputs, since the SPMD graph is identical on all
cores). Core 2b+g emits out rows [1024g : 1024(g+1)] of batch b; the host
reassembles [4, 2048, 1024].

v3: the host pre-transposes x (per-batch x^T) and pre-casts x^T and all
weights to bf16; phase-0 input DMAs are issued from three engine queues
(sync/scalar/gpsimd, ~600ns of sequencer time per dma_start) in
criticality order so the prologue q/k chains start ~10us in. The fill
work (v projections, next-pair q/k chains, output-projection passes) is
balanced across the four attention windows against the 64-step exp
budget; the out-projection runs as pass 1a (pairs 0+1 + bias, hosted in
attention(2)), pass 1b (pair 2, attention(3)), and pass 2 (pair 3 after
the last-pair AllGather chunks + two-stage mask-select). attention(3)
processes i-chunks in order (0,2,1,3) so the cpair-0 select inputs are
both gathered mid-pass and only chunk 3's AG + 8 short chains trail the
last exp. Output is stored bf16 (the host casts to f32).

TensorEngine math is bf16 with f32 PSUM accumulation for attn@v and the
projections. Softmax skips max-subtraction (scaled scores are ~N(0,1));
exp runs on the scalar engine (bf16 PSUM in -> bf16 SBUF out, scale
fused). Denominators come from a ones column appended to v; the PSUM
accumulator is released with two fast copies and normalization
(reciprocal_approx_fast + partition_broadcast + in-place multiply) runs
lazily off the critical path. Score matmuls (K=64) run two heads
concurrently via tile_position row groups. q/k projections for pair p+1
are emitted after attention(p) and v projections inside attention(0)'s
first column loop, so the TensorEngine fills its slack while attention is
ACT(exp)-bound.
"""

import ml_dtypes
import numpy as np

import concourse.mybir as mybir
import concourse.tile as tile
from concourse import bacc, bass_utils

N_CORES = 8
B, N, D = 4, 2048, 1024
GH = 8          # heads per core
DH = 64
IN = GH * DH    # 512 inner dims per core
SCALE = DH ** -0.5
PT = 128
KD = D // PT    # 8 dim tiles
MS = N // PT    # 16 seq tiles
MI = IN // PT   # 4 head-pair tiles per core
NH = N // 2     # out rows per core
F32 = mybir.dt.float32
BF16 = mybir.dt.bfloat16
FP8 = mybir.dt.float8e4
RG = [[0, 1], [2, 3], [4, 5], [6, 7]]

_COMPILED = None


def build():
    nc = bacc.Bacc("TRN2", target_bir_lowering=False, debug=False, num_devices=N_CORES)

    xt_ext = nc.dram_tensor("xt", [D, N], BF16, kind="ExternalInput")
    wq_ext = nc.dram_tensor("wq", [D, IN], BF16, kind="ExternalInput")
    wk_ext = nc.dram_tensor("wk", [D, IN], BF16, kind="ExternalInput")
    wv_ext = nc.dram_tensor("wv", [D, IN], BF16, kind="ExternalInput")
    wout_ext = nc.dram_tensor("wout", [D, D], BF16, kind="ExternalInput")
    bout_ext = nc.dram_tensor("bout", [D], F32, kind="ExternalInput")
    sel_ext = nc.dram_tensor("sel", [1, 2], F32, kind="ExternalInput")
    out_ext = nc.dram_tensor("out", [NH, D], BF16, kind="ExternalOutput")

    with tile.TileContext(nc) as tc:
        with (
            tc.tile_pool(name="const", bufs=1) as constp,
            tc.tile_pool(name="wpool", bufs=1) as wpool,
            tc.tile_pool(name="qkv", bufs=1) as qkv,
            tc.tile_pool(name="attout", bufs=1) as attoutp,
            tc.tile_pool(name="dram", bufs=1, space="DRAM") as dram,
        ):
            ones_col = constp.tile([1, PT], BF16)
            nc.gpsimd.memset(ones_col[:], 1.0)
            # preload the Exp activation table during the DMA-bound head so
            # the first real exp doesn't pay the ~1.3us table load
            dummy = constp.tile([1, 16], F32)
            nc.scalar.activation(
                dummy[:], ones_col[:, 0:16], mybir.ActivationFunctionType.Exp
            )
            bias_row = constp.tile([1, D], F32)
            sel_row = constp.tile([1, 2], F32)
            s0_bc = constp.tile([PT, 1], F32)
            s1_bc = constp.tile([PT, 1], F32)
            # bias broadcast to all partitions: added during the out-proj
            # PSUM->SBUF copies (avoids a K=1 bias matmul per chain)
            bias_st = [constp.tile([PT, 512], F32, name=f"bias_st{i}") for i in range(2)]

            wq_bf = [wpool.tile([PT, IN], BF16, name=f"wq_bf{k}") for k in range(KD)]
            wk_bf = [wpool.tile([PT, IN], BF16, name=f"wk_bf{k}") for k in range(KD)]
            wv_bf = [wpool.tile([PT, IN], BF16, name=f"wv_bf{k}") for k in range(KD)]
            wo_bf = [wpool.tile([PT, D], BF16, name=f"wo_bf{k}") for k in range(KD)]
            xT = [wpool.tile([PT, N], BF16, name=f"xT{k}") for k in range(KD)]

            qT = [qkv.tile([PT, N], BF16, name=f"qT{m}") for m in range(MI)]
            kT = [qkv.tile([PT, N], BF16, name=f"kT{m}") for m in range(MI)]
            vsb = [qkv.tile([PT, GH, 66], BF16, name=f"v{s}") for s in range(MS)]


            attoutT = [attoutp.tile([PT, N], BF16, name=f"attoutT{p}") for p in range(MI)]
            # out-projection pass-1 partial sums (bias + pairs 0-2), staged
            # during attention(3); pass 2 adds pairs 3's contribution at tail
            stage = [attoutp.tile([PT, 512], BF16, name=f"stage{i}") for i in range(16)]
            # persistent select-prefix tiles (s0 * early AG chunk)
            tsel = [attoutp.tile([PT, 512], BF16, name=f"tsel{i}") for i in range(4)]
            # after AG(p) the attoutT data is snapshotted to DRAM; reuse the
            # tile halves for the mask-selected gathered k-tiles kk=p
            # (cols 0:NH) and kk=p+MI (cols NH:N)
            attThalf = [
                attoutT[k % MI][:, (k // MI) * NH:(k // MI + 1) * NH]
                for k in range(KD)
            ]
            ag_in = [dram.tile([PT, N], BF16, name=f"ag_in{p}") for p in range(MI)]
            ag_out = [dram.tile([2 * PT, N], BF16, name=f"ag_out{p}") for p in range(MI)]
            ag_chunk = [dram.tile([2 * PT, 512], BF16, name=f"ag_chunk{i}") for i in range(4)]
            ag_cin = [dram.tile([PT, 512], BF16, name=f"ag_cin{i}") for i in range(4)]

            # ================= phase 0: direct bf16 loads ==============
            # criticality order: pair-0 columns of wk/wq + xT ch0 gate the
            # prologue chains (~4us in); wv gates vproj (fill0 iq0, ~12us);
            # xT ch1..3 gate k0-ch1..3 (steps j=4/8/12); the remaining
            # wq/wk columns are first needed at attention(0) step 16.
            # dma_start costs ~600ns of sequencer time each; spread the 82
            # input DMAs across four engine queues so issue parallelizes.
            # Criticality: wk/wq pair-0 cols + x-ch0 + wv gate the prologue
            # chains and the first v projections.
            for k in range(KD):
                nc.sync.dma_start(wk_bf[k][:, 0:PT], wk_ext[k * PT:(k + 1) * PT, 0:PT])
            for k in range(KD):
                nc.scalar.dma_start(wq_bf[k][:, 0:PT], wq_ext[k * PT:(k + 1) * PT, 0:PT])
            for k in range(KD):
                nc.gpsimd.dma_start(
                    xT[k][:, 0:512], xt_ext[k * PT:(k + 1) * PT, 0:512]
                )
            for k in range(KD):
                nc.gpsimd.dma_start(wv_bf[k][:], wv_ext[k * PT:(k + 1) * PT, :])
            for k in range(KD):
                nc.sync.dma_start(
                    xT[k][:, 512:1024], xt_ext[k * PT:(k + 1) * PT, 512:1024]
                )
            for k in range(KD):
                nc.sync.dma_start(
                    xT[k][:, 1024:1536], xt_ext[k * PT:(k + 1) * PT, 1024:1536]
                )
            for k in range(KD):
                nc.sync.dma_start(
                    xT[k][:, 1536:2048], xt_ext[k * PT:(k + 1) * PT, 1536:2048]
                )
            for k in range(KD):
                nc.sync.dma_start(wk_bf[k][:, PT:IN], wk_ext[k * PT:(k + 1) * PT, PT:IN])
            for k in range(KD):
                nc.sync.dma_start(wq_bf[k][:, PT:IN], wq_ext[k * PT:(k + 1) * PT, PT:IN])
            for k in range(KD):
                nc.sync.dma_start(wo_bf[k][:], wout_ext[k * PT:(k + 1) * PT, :])
            nc.sync.dma_start(bias_row[:], bout_ext[None, :])
            nc.sync.dma_start(sel_row[:], sel_ext[:])
            nc.gpsimd.partition_broadcast(s0_bc[:], sel_row[:, 0:1])
            nc.gpsimd.partition_broadcast(s1_bc[:], sel_row[:, 1:2])
            for i in range(2):
                nc.gpsimd.partition_broadcast(
                    bias_st[i][:], bias_row[:, i * 512:(i + 1) * 512]
                )

            # prologue: qkproj(0) with a 3-deep PSUM pool (the attention pools
            # are not open yet) so the 8 chains pipeline instead of
            # serializing on a single bank
            # only the chains attention(0) needs early: k-ch0 + q-ch0 unblock
            # the first scores, k-ch1..3 unblock j=4..15, q-ch1 unblocks
            # iq=1; q-ch2/3 (consumed at steps 32/48) fill inside attention(0)
            def vproj_s(s, pool, tag):
                pv = pool.tile([PT, 512], F32, name="pv", tag=tag)
                for k in range(KD):
                    nc.tensor.matmul(
                        pv[:],
                        xT[k][:, s * PT:(s + 1) * PT],
                        wv_bf[k][:],
                        start=(k == 0), stop=(k == KD - 1),
                    )
                nc.gpsimd.memset(vsb[s][:, :, 64:65], 1.0)
                nc.vector.tensor_copy(
                    vsb[s][:, :, 0:64],
                    pv[:].rearrange("p (h e) -> p h e", h=GH),
                )

            with (
                tc.tile_pool(name="psP", bufs=1, space="PSUM") as psP,
                tc.tile_pool(name="agst", bufs=2) as agst,
                tc.tile_pool(name="osb", bufs=4) as osbp,
            ):
                # ============ phases 1+2 interleaved ==============
                with (
                    tc.tile_pool(name="psS", bufs=2, space="PSUM") as psS,
                    tc.tile_pool(name="psO", bufs=3, space="PSUM") as psO,
                    tc.tile_pool(name="attn", bufs=5) as attnp,
                    tc.tile_pool(name="fin", bufs=1) as finp,
                ):
                    def ag_full(p):
                        nc.sync.dma_start(ag_in[p][:], attoutT[p][:])
                        nc.gpsimd.collective_compute(
                            "AllGather", mybir.AluOpType.bypass,
                            replica_groups=RG,
                            ins=[ag_in[p].opt()], outs=[ag_out[p].opt()],
                        )

                    def ag_iq(p, iq):
                        cs = iq * 512
                        nc.sync.dma_start(
                            ag_cin[iq][:], attoutT[p][:, cs:cs + 512]
                        )
                        nc.gpsimd.collective_compute(
                            "AllGather", mybir.AluOpType.bypass,
                            replica_groups=RG,
                            ins=[ag_cin[iq].opt()],
                            outs=[ag_chunk[iq].opt()],
                        )

                    def attention(p, fill=None, iq_order=(0, 1, 2, 3)):
                        last = p == MI - 1
                        # flat software pipeline over (iq, j): v-matmuls
                        # run one step behind S/exp so the next block's
                        # score matmul never queues behind exp-gated work
                        steps = [(iq, j) for iq in iq_order for j in range(MS)]
                        outs = {}
                        pend = None  # (iq, j, at)
                        for iq, j in steps:
                            if j == 0:
                                outs[iq] = (
                                    psO.tile([65, 512], F32, name="oA", tag="psO"),
                                    psO.tile([65, 512], F32, name="oB", tag="psO"),
                                )
                            ps = psS.tile([PT, 1024], F32, name="ps", tag="psS")
                            nc.tensor.matmul(
                                ps[:, 0:512],
                                kT[p][0:64, j * PT:(j + 1) * PT],
                                qT[p][0:64, iq * 512:(iq + 1) * 512],
                                start=True, stop=True,
                                tile_position=(0, 0),
                            )
                            nc.tensor.matmul(
                                ps[:, 512:1024],
                                kT[p][64:128, j * PT:(j + 1) * PT],
                                qT[p][64:128, iq * 512:(iq + 1) * 512],
                                start=True, stop=True,
                                tile_position=(64, 0),
                            )
                            at = attnp.tile([PT, 1024], BF16, name="at", tag="at")
                            nc.scalar.activation(
                                at[:], ps[:], mybir.ActivationFunctionType.Exp,
                                scale=SCALE,
                            )
                            if fill is not None:
                                fill(iq, j)
                            if pend is not None:
                                self_emit_vmm(p, outs, *pend)
                                if pend[1] == MS - 1:
                                    self_finalize(p, outs, pend[0], last)
                            pend = (iq, j, at)
                        self_emit_vmm(p, outs, *pend)
                        self_finalize(p, outs, pend[0], last)
                        if not last:
                            ag_full(p)
                            # stage + mask-select this pair's two gathered
                            # k-tiles. On gpsimd: a late AG then only stalls
                            # the (slack-rich) gpsimd queue, not the vector
                            # queue that carries the finalize copies.
                            for kk in (p, p + MI):
                                half = kk // MI
                                astA = agst.tile([PT, NH], BF16, name="astA",
                                                 tag="ast")
                                nc.sync.dma_start(
                                    astA[:],
                                    ag_out[p][half * PT:(half + 1) * PT, 0:NH],
                                )
                                tmp = agst.tile([PT, NH], BF16, name="tmp", tag="tmp")
                                nc.vector.tensor_scalar_mul(
                                    tmp[:], astA[:], s0_bc[:]
                                )
                                astB = agst.tile([PT, NH], BF16, name="astB",
                                                 tag="ast")
                                nc.sync.dma_start(
                                    astB[:],
                                    ag_out[p][half * PT:(half + 1) * PT, NH:N],
                                )
                                nc.vector.scalar_tensor_tensor(
                                    attThalf[kk],
                                    astB[:], s1_bc[:], tmp[:],
                                    op0=mybir.AluOpType.mult,
                                    op1=mybir.AluOpType.add,
                                )

                    # last-pair select, split in two stages so only stage B
                    # (one op per kk) depends on the later-arriving chunk:
                    # attThalf[kk][:, cpair*512:+512] =
                    #     s0*ag_chunk[cpair] + s1*ag_chunk[cpair+2]
                    def sel_stageA(cpair):
                        for ki, kk in enumerate((3, 7)):
                            half = kk // MI
                            a0 = agst.tile([PT, 512], BF16, name="a0", tag="a0")
                            nc.sync.dma_start(
                                a0[:], ag_chunk[cpair][half * PT:(half + 1) * PT, :]
                            )
                            nc.vector.tensor_scalar_mul(
                                tsel[cpair * 2 + ki][:], a0[:], s0_bc[:]
                            )

                    def sel_stageB(cpair):
                        cs = cpair * 512
                        for ki, kk in enumerate((3, 7)):
                            half = kk // MI
                            a2 = agst.tile([PT, 512], BF16, name="a2", tag="a2")
                            eng = nc.scalar if (cpair == 1 and ki == 1) else nc.sync
                            eng.dma_start(
                                a2[:], ag_chunk[cpair + 2][half * PT:(half + 1) * PT, :]
                            )
                            nc.vector.scalar_tensor_tensor(
                                attThalf[kk][:, cs:cs + 512],
                                a2[:], s1_bc[:], tsel[cpair * 2 + ki][:],
                                op0=mybir.AluOpType.mult,
                                op1=mybir.AluOpType.add,
                            )

                    def self_emit_vmm(p, outs, iq, j, at):
                        oA, oB = outs[iq]
                        nc.tensor.matmul(
                            oA[:], vsb[j][:, 2 * p, 0:65], at[:, 0:512],
                            start=(j == 0), stop=(j == MS - 1),
                        )
                        nc.tensor.matmul(
                            oB[:], vsb[j][:, 2 * p + 1, 0:65], at[:, 512:1024],
                            start=(j == 0), stop=(j == MS - 1),
                        )

                    def self_finalize(p, outs, iq, last):
                        # on the very last block the scalar engine is idle and
                        # these copies gate the final AllGather chunk: use it
                        tail = last and iq == 3
                        # both heads' denominators share one tile so the
                        # normalization is a single recip + single broadcast
                        den = finp.tile([1, 1024], F32, name="den", tag="den")
                        for hh, o in enumerate(outs[iq]):
                            seg = attoutT[p][hh * 64:(hh + 1) * 64,
                                             iq * 512:(iq + 1) * 512]
                            dslice = den[:, hh * 512:(hh + 1) * 512]
                            if tail and hh == 0:
                                # split the last block's copies across both
                                # engines: scalar is idle once exps finish
                                nc.scalar.copy(seg, o[0:64, :])
                                nc.scalar.copy(dslice, o[64:65, :])
                            else:
                                nc.vector.tensor_copy(seg, o[0:64, :])
                                nc.vector.tensor_copy(dslice, o[64:65, :])
                        recip = finp.tile([1, 1024], F32, name="recip",
                                          tag="recip")
                        nc.vector.reciprocal_approx_fast(recip[:], den[:])
                        bc = finp.tile([PT, 1024], F32, name="bc", tag="bc")
                        nc.gpsimd.partition_broadcast(bc[:], recip[:])
                        for hh in range(2):
                            seg = attoutT[p][hh * 64:(hh + 1) * 64,
                                             iq * 512:(iq + 1) * 512]
                            nc.vector.tensor_tensor(
                                seg, seg,
                                bc[hh * 64:(hh + 1) * 64,
                                   hh * 512:(hh + 1) * 512],
                                op=mybir.AluOpType.mult,
                            )
                        if last:
                            ag_iq(p, iq)

                    # q/k projections for pair m as half-chains in the hosting
                    # attention's per-step slack (replaces the standalone
                    # qkproj bursts at pair boundaries). k-chains first: kT[m]
                    # is consumed across ALL column chunks from step ~12 of
                    # attention(m), while qT[m] chunk iq is needed only when
                    # its iq block starts.
                    qk_chain = {}

                    def half_chain(m, wsel, ch, half):
                        w_bf, dstT = ((wk_bf, kT), (wq_bf, qT))[wsel]
                        key = (m, wsel, ch)
                        if half == 0:
                            ph = psP.tile([PT, 512], F32, name="ph", tag="psP")
                            qk_chain[key] = ph
                            ks = range(0, KD // 2)
                        else:
                            ph = qk_chain.pop(key)
                            ks = range(KD // 2, KD)
                        for k in ks:
                            nc.tensor.matmul(
                                ph[:],
                                w_bf[k][:, m * PT:(m + 1) * PT],
                                xT[k][:, ch * 512:(ch + 1) * 512],
                                start=(k == 0), stop=(k == KD - 1),
                            )
                        if half == 1:
                            nc.vector.tensor_copy(
                                dstT[m][:, ch * 512:(ch + 1) * 512], ph[:]
                            )

                    def chain_fill(sched, s0, stride, pre=None):
                        # sched: list of (m, wsel, ch) chains, emitted as
                        # half-chains every `stride` steps from step s0
                        def f(iq, j):
                            s = iq * MS + j
                            if pre is not None:
                                pre(iq, j)
                            if s < s0 or (s - s0) % stride != 0:
                                return
                            hc = (s - s0) // stride
                            if hc < 2 * len(sched):
                                c, half = divmod(hc, 2)
                                half_chain(*sched[c], half)
                        return f

                    # global fill balance (per-window TensorE budget is
                    # 64 exp-steps ~= 171k cycles; own scores+vmm = 98k):
                    #   att(0): vproj 6..15 + q0-ch2/3 + k1-all + q1-ch0
                    #   att(1): q1-ch1..3 + all of pair 2 + k3-all + q3-ch0
                    #   att(2): q3-ch1..3 + out-proj pass 1a
                    #   att(3): pass 1b + selects + pass 2a
                    sched0 = [(0, 1, 2), (0, 1, 3),
                              (1, 0, 0), (1, 0, 1), (1, 0, 2), (1, 0, 3),
                              (1, 1, 0)]

                    def pre0(iq, j):
                        if iq == 0:
                            vproj_s(j, psP, "psP")

                    fill0 = chain_fill(sched0, MS, 2, pre=pre0)

                    sched1 = [(1, 1, 1), (1, 1, 2), (1, 1, 3),
                              (2, 0, 0), (2, 1, 0), (2, 0, 1), (2, 0, 2),
                              (2, 0, 3), (2, 1, 1), (2, 1, 2), (2, 1, 3),
                              (3, 0, 0), (3, 0, 1), (3, 0, 2), (3, 0, 3),
                              (3, 1, 0)]
                    fill1 = chain_fill(sched1, 0, 2)

                    sched2 = [(3, 1, 1), (3, 1, 2), (3, 1, 3)]

                    def fill2(iq, j):
                        s = iq * MS + j
                        if s < 6:
                            c, half = divmod(s, 2)
                            half_chain(*sched2[c], half)
                        elif 12 <= s < 44 and (s - 12) % 2 == 0:
                            p1a((s - 12) // 2)

                    # out-projection pass 1a (pairs 0+1 + bias -> stage,
                    # hosted in attention(2)) and 1b (pair 2 added in-place,
                    # hosted in attention(3))
                    def p1a(t):
                        m, nn = divmod(t, 2)
                        pp = psP.tile([PT, 512], F32, name="pp", tag="psP")
                        for ki, kk in enumerate((0, 4, 1, 5)):
                            nc.tensor.matmul(
                                pp[:], attThalf[kk][:, m * PT:(m + 1) * PT],
                                wo_bf[kk][:, nn * 512:(nn + 1) * 512],
                                start=(ki == 0), stop=(ki == 3),
                            )
                        nc.vector.tensor_tensor(
                            stage[t][:], pp[:], bias_st[nn][:],
                            op=mybir.AluOpType.add,
                        )

                    def p1b(t):
                        m, nn = divmod(t, 2)
                        pp = psP.tile([PT, 512], F32, name="pb", tag="psP")
                        for ki, kk in enumerate((2, 6)):
                            nc.tensor.matmul(
                                pp[:], attThalf[kk][:, m * PT:(m + 1) * PT],
                                wo_bf[kk][:, nn * 512:(nn + 1) * 512],
                                start=(ki == 0), stop=(ki == 1),
                            )
                        nc.vector.tensor_tensor(
                            stage[t][:], pp[:], stage[t][:],
                            op=mybir.AluOpType.add,
                        )

                    # pass 2: adds pair-3's contribution (attThalf[3]/[7]) to
                    # the staged pass-1 partials and stores bf16 output rows
                    p2_chains = {}

                    def p2_half(m, nn, half, pool=None, tail=False):
                        t = m * 2 + nn
                        if half == 0:
                            pool = pool if pool is not None else psP
                            pp = pool.tile([PT, 512], F32, name="p2", tag="psP")
                            p2_chains[t] = pp
                            nc.tensor.matmul(
                                pp[:], attThalf[3][:, m * PT:(m + 1) * PT],
                                wo_bf[3][:, nn * 512:(nn + 1) * 512],
                                start=True, stop=False,
                            )
                        else:
                            pp = p2_chains.pop(t)
                            nc.tensor.matmul(
                                pp[:], attThalf[7][:, m * PT:(m + 1) * PT],
                                wo_bf[7][:, nn * 512:(nn + 1) * 512],
                                start=False, stop=True,
                            )
                            osb = osbp.tile([PT, 512], BF16, name="osb", tag="osb")
                            nc.vector.tensor_tensor(
                                osb[:], pp[:], stage[t][:],
                                op=mybir.AluOpType.add,
                            )
                            eng = nc.scalar if tail else nc.sync
                            eng.dma_start(
                                out_ext[m * PT:(m + 1) * PT,
                                        nn * 512:(nn + 1) * 512],
                                osb[:],
                            )

                    # attention(3) i-chunk order (0,2,1,3): chunks 0+2 (the
                    # cpair-0 select inputs) are both gathered by mid-pass, so
                    # the m0..3 output rows complete inside the attention
                    # window; only chunk 3's AG + m4..7 remain at the tail.
                    ORD3 = (0, 2, 1, 3)

                    def fill3(iq, j):
                        si = ORD3.index(iq) * MS + j
                        if 8 <= si < 40 and (si - 8) % 2 == 0:
                            p1b((si - 8) // 2)
                        if si == 20:
                            sel_stageA(0)
                        elif si == 40:
                            sel_stageB(0)
                        elif si == 58:
                            sel_stageA(1)

                    # prologue: pair-0 q/k chains through the psO slots
                    # (scores/exp use psS/psP and start as soon as the
                    # first two chains land; the vmm accumulators naturally
                    # queue behind the remaining chains)
                    # five chains through the psO slots; q0-ch1 (needed
                    # only at step 16) goes through psP so the second vmm
                    # accumulator slot frees one chain earlier
                    for pool, w_bf, dstT, ch in (
                        (psO, wk_bf, kT, 0), (psO, wq_bf, qT, 0),
                        (psO, wk_bf, kT, 1), (psO, wk_bf, kT, 2),
                        (psO, wk_bf, kT, 3), (psP, wq_bf, qT, 1),
                    ):
                        tag = "psO" if pool is psO else "psP"
                        pq = pool.tile([PT, 512], F32, name="pq", tag=tag)
                        for k in range(KD):
                            nc.tensor.matmul(
                                pq[:],
                                w_bf[k][:, 0:PT],
                                xT[k][:, ch * 512:(ch + 1) * 512],
                                start=(k == 0), stop=(k == KD - 1),
                            )
                        nc.vector.tensor_copy(
                            dstT[0][:, ch * 512:(ch + 1) * 512], pq[:]
                        )

                    attention(0, fill=fill0)
                    attention(1, fill=fill1)
                    attention(2, fill=fill2)
                    attention(3, fill=fill3, iq_order=ORD3)

                # ==== tail ====
                # pass-2a (m0..3, select already done mid-window) is emitted
                # first so its matmuls/adds/stores hide under chunk 3's
                # AllGather flight; then the AG-gated cpair-1 select + m4..7
                with tc.tile_pool(name="pso", bufs=4, space="PSUM") as pso_p:
                    for m in range(0, 4):
                        for nn in range(2):
                            p2_half(m, nn, 0, pool=pso_p)
                            p2_half(m, nn, 1)
                    sel_stageB(1)
                    for m in range(4, 8):
                        for nn in range(2):
                            p2_half(m, nn, 0, pool=pso_p)
                            p2_half(m, nn, 1, tail=True)

    nc.compile()
    return nc


def _shard_inputs(x, Wq, Wkv, Wout, bout):
    BF = ml_dtypes.bfloat16
    x = np.asarray(x, dtype=np.float32)
    wq_g = [np.ascontiguousarray(Wq[:, g * IN:(g + 1) * IN]).astype(BF) for g in range(2)]
    wk_g = [np.ascontiguousarray(Wkv[:, g * IN:(g + 1) * IN]).astype(BF) for g in range(2)]
    wv_g = [
        np.ascontiguousarray(Wkv[:, D + g * IN:D + (g + 1) * IN]).astype(BF)
        for g in range(2)
    ]
    wo = np.ascontiguousarray(Wout).astype(BF)
    bo = np.ascontiguousarray(bout, dtype=np.float32)
    xt_b = [np.ascontiguousarray(x[b].T).astype(BF) for b in range(B)]
    in_maps = []
    for c in range(N_CORES):
        b, g = c // 2, c % 2
        sel = np.zeros((1, 2), dtype=np.float32)
        sel[0, g] = 1.0
        in_maps.append({
            "xt": xt_b[b],
            "wq": wq_g[g],
            "wk": wk_g[g],
            "wv": wv_g[g],
            "wout": wo,
            "bout": bo,
            "sel": sel,
        })
    return in_maps


def kernel(x, Wq, Wkv, Wout, bout):
    global _COMPILED
    if _COMPILED is None:
        _COMPILED = build()
    nc = _COMPILED
    in_maps = _shard_inputs(
        np.asarray(x), np.asarray(Wq), np.asarray(Wkv), np.asarray(Wout),
        np.asarray(bout),
    )
    res = bass_utils.run_bass_kernel_spmd(nc, in_maps, core_ids=list(range(N_CORES)))
    out = np.empty((B, N, D), dtype=np.float32)
    for c in range(N_CORES):
        b, g = c // 2, c % 2
        out[b, g * NH:(g + 1) * NH, :] = np.asarray(
            res.results[c]["out"], dtype=np.float32
        )
    return out


if __name__ == "__main__":
    rng = np.random.default_rng(0)
    x = rng.standard_normal((B, N, D)).astype(np.float32)
    Wq = rng.standard_normal((D, D)).astype(np.float32) * D ** -0.5
    Wkv = rng.standard_normal((D, 2 * D)).astype(np.float32) * D ** -0.5
    Wout = rng.standard_normal((D, D)).astype(np.float32) * D ** -0.5
    bout = np.zeros((D,), dtype=np.float32)
    y = kernel(x=x, Wq=Wq, Wkv=Wkv, Wout=Wout, bout=bout)
    print("out shape:", y.shape, "finite:", np.isfinite(y).all())



# revision 56
# speedup vs baseline: 1.0024x; 1.0024x over previous
"""Distributed multi-head self-attention for Trainium2 (8 NeuronCores).

Problem: b=4, n=2048, dim=1024, heads=16, dim_head=64.
  q = x@Wq; k,v = split(x@Wkv, 2); out = softmax(q k^T / 8) v; y = out@Wout + bout

Sharding: core c <-> (batch b=c//2, head-group g=c%2). Each core computes
q/k/v + attention for its batch's 8 heads (tensor-parallel columns of
Wq/Wkv). The pair (b,0)/(b,1) AllGathers the transposed bf16 attention
outputs (per head-pair, overlapped with attention compute; the last pair
streams per-i-chunk), then each core runs the output projection with the
full Wout over ITS HALF of the sequence (selected from the gathered buffer
with per-core one-hot mask inputs, since the SPMD graph is identical on all
cores). Core 2b+g emits out rows [1024g : 1024(g+1)] of batch b; the host
reassembles [4, 2048, 1024].

v3: the host pre-transposes x (per-batch x^T) and pre-casts x^T and all
weights to bf16; phase-0 input DMAs are issued from three engine queues
(sync/scalar/gpsimd, ~600ns of sequencer time per dma_start) in
criticality order so the prologue q/k chains start ~10us in. The fill
work (v projections, next-pair q/k chains, output-projection passes) is
balanced across the four attention windows against the 64-step exp
budget; the out-projection runs as pass 1a (pairs 0+1 + bias, hosted in
attention(2)), pass 1b (pair 2, attention(3)), and pass 2 (pair 3 after
the last-pair AllGather chunks + two-stage mask-select). attention(3)
processes i-chunks in order (0,2,1,3) so the cpair-0 select inputs are
both gathered mid-pass and only chunk 3's AG + 8 short chains trail the
last exp. Output is stored bf16 (the host casts to f32).

TensorEngine math is bf16 with f32 PSUM accumulation for attn@v and the
projections. Softmax skips max-subtraction (scaled scores are ~N(0,1));
exp runs on the scalar engine (bf16 PSUM in -> bf16 SBUF out, scale
fused). Denominators come from a ones column appended to v; the PSUM
accumulator is released with two fast copies and normalization
(reciprocal_approx_fast + partition_broadcast + in-place multiply) runs
lazily off the critical path. Score matmuls (K=64) run two heads
concurrently via tile_position row groups. q/k projections for pair p+1
are emitted after attention(p) and v projections inside attention(0)'s
first column loop, so the TensorEngine fills its slack while attention is
ACT(exp)-bound.
"""

import ml_dtypes
import numpy as np

import concourse.mybir as mybir
import concourse.tile as tile
from concourse import bacc, bass_utils

N_CORES = 8
B, N, D = 4, 2048, 1024
GH = 8          # heads per core
DH = 64
IN = GH * DH    # 512 inner dims per core
SCALE = DH ** -0.5
PT = 128
KD = D // PT    # 8 dim tiles
MS = N // PT    # 16 seq tiles
MI = IN // PT   # 4 head-pair tiles per core
NH = N // 2     # out rows per core
F32 = mybir.dt.float32
BF16 = mybir.dt.bfloat16
FP8 = mybir.dt.float8e4
RG = [[0, 1], [2, 3], [4, 5], [6, 7]]

_COMPILED = None


def build():
    nc = bacc.Bacc("TRN2", target_bir_lowering=False, debug=False, num_devices=N_CORES)

    xt_ext = nc.dram_tensor("xt", [D, N], BF16, kind="ExternalInput")
    wq_ext = nc.dram_tensor("wq", [D, IN], BF16, kind="ExternalInput")
    wk_ext = nc.dram_tensor("wk", [D, IN], BF16, kind="ExternalInput")
    wv_ext = nc.dram_tensor("wv", [D, IN], BF16, kind="ExternalInput")
    wout_ext = nc.dram_tensor("wout", [D, D], BF16, kind="ExternalInput")
    bout_ext = nc.dram_tensor("bout", [D], F32, kind="ExternalInput")
    sel_ext = nc.dram_tensor("sel", [1, 2], F32, kind="ExternalInput")
    out_ext = nc.dram_tensor("out", [NH, D], BF16, kind="ExternalOutput")

    with tile.TileContext(nc) as tc:
        with (
            tc.tile_pool(name="const", bufs=1) as constp,
            tc.tile_pool(name="wpool", bufs=1) as wpool,
            tc.tile_pool(name="qkv", bufs=1) as qkv,
            tc.tile_pool(name="attout", bufs=1) as attoutp,
            tc.tile_pool(name="dram", bufs=1, space="DRAM") as dram,
        ):
            ones_col = constp.tile([1, PT], BF16)
            nc.gpsimd.memset(ones_col[:], 1.0)
            # preload the Exp activation table during the DMA-bound head so
            # the first real exp doesn't pay the ~1.3us table load
            dummy = constp.tile([1, 16], F32)
            nc.scalar.activation(
                dummy[:], ones_col[:, 0:16], mybir.ActivationFunctionType.Exp
            )
            bias_row = constp.tile([1, D], F32)
            sel_row = constp.tile([1, 2], F32)
            s0_bc = constp.tile([PT, 1], F32)
            s1_bc = constp.tile([PT, 1], F32)
            # bias broadcast to all partitions: added during the out-proj
            # PSUM->SBUF copies (avoids a K=1 bias matmul per chain)
            bias_st = [constp.tile([PT, 512], F32, name=f"bias_st{i}") for i in range(2)]

            wq_bf = [wpool.tile([PT, IN], BF16, name=f"wq_bf{k}") for k in range(KD)]
            wk_bf = [wpool.tile([PT, IN], BF16, name=f"wk_bf{k}") for k in range(KD)]
            wv_bf = [wpool.tile([PT, IN], BF16, name=f"wv_bf{k}") for k in range(KD)]
            wo_bf = [wpool.tile([PT, D], BF16, name=f"wo_bf{k}") for k in range(KD)]
            xT = [wpool.tile([PT, N], BF16, name=f"xT{k}") for k in range(KD)]

            qT = [qkv.tile([PT, N], BF16, name=f"qT{m}") for m in range(MI)]
            kT = [qkv.tile([PT, N], BF16, name=f"kT{m}") for m in range(MI)]
            vsb = [qkv.tile([PT, GH, 66], BF16, name=f"v{s}") for s in range(MS)]


            attoutT = [attoutp.tile([PT, N], BF16, name=f"attoutT{p}") for p in range(MI)]
            # out-projection pass-1 partial sums (bias + pairs 0-2), staged
            # during attention(3); pass 2 adds pairs 3's contribution at tail
            stage = [attoutp.tile([PT, 512], BF16, name=f"stage{i}") for i in range(16)]
            # persistent select-prefix tiles (s0 * early AG chunk)
            tsel = [attoutp.tile([PT, 512], BF16, name=f"tsel{i}") for i in range(4)]
            # after AG(p) the attoutT data is snapshotted to DRAM; reuse the
            # tile halves for the mask-selected gathered k-tiles kk=p
            # (cols 0:NH) and kk=p+MI (cols NH:N)
            attThalf = [
                attoutT[k % MI][:, (k // MI) * NH:(k // MI + 1) * NH]
                for k in range(KD)
            ]
            ag_in = [dram.tile([PT, N], BF16, name=f"ag_in{p}") for p in range(MI)]
            ag_out = [dram.tile([2 * PT, N], BF16, name=f"ag_out{p}") for p in range(MI)]
            ag_chunk = [dram.tile([2 * PT, 512], BF16, name=f"ag_chunk{i}") for i in range(4)]
            ag_cin = [dram.tile([PT, 512], BF16, name=f"ag_cin{i}") for i in range(4)]

            # ================= phase 0: direct bf16 loads ==============
            # criticality order: pair-0 columns of wk/wq + xT ch0 gate the
            # prologue chains (~4us in); wv gates vproj (fill0 iq0, ~12us);
            # xT ch1..3 gate k0-ch1..3 (steps j=4/8/12); the remaining
            # wq/wk columns are first needed at attention(0) step 16.
            # dma_start costs ~600ns of sequencer time each; spread the 82
            # input DMAs across four engine queues so issue parallelizes.
            # Criticality: wk/wq pair-0 cols + x-ch0 + wv gate the prologue
            # chains and the first v projections.
            for k in range(KD):
                nc.sync.dma_start(wk_bf[k][:, 0:PT], wk_ext[k * PT:(k + 1) * PT, 0:PT])
            for k in range(KD):
                nc.scalar.dma_start(wq_bf[k][:, 0:PT], wq_ext[k * PT:(k + 1) * PT, 0:PT])
            for k in range(KD):
                nc.gpsimd.dma_start(
                    xT[k][:, 0:512], xt_ext[k * PT:(k + 1) * PT, 0:512]
                )
            for k in range(KD):
                nc.gpsimd.dma_start(wv_bf[k][:], wv_ext[k * PT:(k + 1) * PT, :])
            for k in range(KD):
                nc.sync.dma_start(
                    xT[k][:, 512:1024], xt_ext[k * PT:(k + 1) * PT, 512:1024]
                )
            for k in range(KD):
                nc.sync.dma_start(
                    xT[k][:, 1024:1536], xt_ext[k * PT:(k + 1) * PT, 1024:1536]
                )
            for k in range(KD):
                nc.sync.dma_start(
                    xT[k][:, 1536:2048], xt_ext[k * PT:(k + 1) * PT, 1536:2048]
                )
            for k in range(KD):
                nc.sync.dma_start(wk_bf[k][:, PT:IN], wk_ext[k * PT:(k + 1) * PT, PT:IN])
            for k in range(KD):
                nc.sync.dma_start(wq_bf[k][:, PT:IN], wq_ext[k * PT:(k + 1) * PT, PT:IN])
            for k in range(KD):
                nc.sync.dma_start(wo_bf[k][:], wout_ext[k * PT:(k + 1) * PT, :])
            nc.sync.dma_start(bias_row[:], bout_ext[None, :])
            nc.sync.dma_start(sel_row[:], sel_ext[:])
            nc.gpsimd.partition_broadcast(s0_bc[:], sel_row[:, 0:1])
            nc.gpsimd.partition_broadcast(s1_bc[:], sel_row[:, 1:2])
            for i in range(2):
                nc.gpsimd.partition_broadcast(
                    bias_st[i][:], bias_row[:, i * 512:(i + 1) * 512]
                )

            # prologue: qkproj(0) with a 3-deep PSUM pool (the attention pools
            # are not open yet) so the 8 chains pipeline instead of
            # serializing on a single bank
            # only the chains attention(0) needs early: k-ch0 + q-ch0 unblock
            # the first scores, k-ch1..3 unblock j=4..15, q-ch1 unblocks
            # iq=1; q-ch2/3 (consumed at steps 32/48) fill inside attention(0)
            def vproj_s(s, pool, tag):
                pv = pool.tile([PT, 512], F32, name="pv", tag=tag)
                for k in range(KD):
                    nc.tensor.matmul(
                        pv[:],
                        xT[k][:, s * PT:(s + 1) * PT],
                        wv_bf[k][:],
                        start=(k == 0), stop=(k == KD - 1),
                    )
                nc.gpsimd.memset(vsb[s][:, :, 64:65], 1.0)
                nc.vector.tensor_copy(
                    vsb[s][:, :, 0:64],
                    pv[:].rearrange("p (h e) -> p h e", h=GH),
                )

            with (
                tc.tile_pool(name="psP", bufs=1, space="PSUM") as psP,
                tc.tile_pool(name="agst", bufs=2) as agst,
                tc.tile_pool(name="osb", bufs=4) as osbp,
            ):
                # ============ phases 1+2 interleaved ==============
                with (
                    tc.tile_pool(name="psS", bufs=2, space="PSUM") as psS,
                    tc.tile_pool(name="psO", bufs=3, space="PSUM") as psO,
                    tc.tile_pool(name="attn", bufs=5) as attnp,
                    tc.tile_pool(name="fin", bufs=1) as finp,
                ):
                    def ag_full(p):
                        nc.sync.dma_start(ag_in[p][:], attoutT[p][:])
                        nc.gpsimd.collective_compute(
                            "AllGather", mybir.AluOpType.bypass,
                            replica_groups=RG,
                            ins=[ag_in[p].opt()], outs=[ag_out[p].opt()],
                        )

                    def ag_iq(p, iq):
                        cs = iq * 512
                        nc.sync.dma_start(
                            ag_cin[iq][:], attoutT[p][:, cs:cs + 512]
                        )
                        nc.gpsimd.collective_compute(
                            "AllGather", mybir.AluOpType.bypass,
                            replica_groups=RG,
                            ins=[ag_cin[iq].opt()],
                            outs=[ag_chunk[iq].opt()],
                        )

                    def attention(p, fill=None, iq_order=(0, 1, 2, 3)):
                        last = p == MI - 1
                        # flat software pipeline over (iq, j): v-matmuls
                        # run one step behind S/exp so the next block's
                        # score matmul never queues behind exp-gated work
                        steps = [(iq, j) for iq in iq_order for j in range(MS)]
                        outs = {}
                        pend = None  # (iq, j, at)
                        for iq, j in steps:
                            if j == 0:
                                outs[iq] = (
                                    psO.tile([65, 512], F32, name="oA", tag="psO"),
                                    psO.tile([65, 512], F32, name="oB", tag="psO"),
                                )
                            ps = psS.tile([PT, 1024], F32, name="ps", tag="psS")
                            nc.tensor.matmul(
                                ps[:, 0:512],
                                kT[p][0:64, j * PT:(j + 1) * PT],
                                qT[p][0:64, iq * 512:(iq + 1) * 512],
                                start=True, stop=True,
                                tile_position=(0, 0),
                            )
                            nc.tensor.matmul(
                                ps[:, 512:1024],
                                kT[p][64:128, j * PT:(j + 1) * PT],
                                qT[p][64:128, iq * 512:(iq + 1) * 512],
                                start=True, stop=True,
                                tile_position=(64, 0),
                            )
                            at = attnp.tile([PT, 1024], BF16, name="at", tag="at")
                            nc.scalar.activation(
                                at[:], ps[:], mybir.ActivationFunctionType.Exp,
                                scale=SCALE,
                            )
                            if fill is not None:
                                fill(iq, j)
                            if pend is not None:
                                self_emit_vmm(p, outs, *pend)
                                if pend[1] == MS - 1:
                                    self_finalize(p, outs, pend[0], last)
                            pend = (iq, j, at)
                        self_emit_vmm(p, outs, *pend)
                        self_finalize(p, outs, pend[0], last)
                        if not last:
                            ag_full(p)
                            # stage + mask-select this pair's two gathered
                            # k-tiles. On gpsimd: a late AG then only stalls
                            # the (slack-rich) gpsimd queue, not the vector
                            # queue that carries the finalize copies.
                            for kk in (p, p + MI):
                                half = kk // MI
                                astA = agst.tile([PT, NH], BF16, name="astA",
                                                 tag="ast")
                                nc.sync.dma_start(
                                    astA[:],
                                    ag_out[p][half * PT:(half + 1) * PT, 0:NH],
                                )
                                tmp = agst.tile([PT, NH], BF16, name="tmp", tag="tmp")
                                nc.vector.tensor_scalar_mul(
                                    tmp[:], astA[:], s0_bc[:]
                                )
                                astB = agst.tile([PT, NH], BF16, name="astB",
                                                 tag="ast")
                                nc.sync.dma_start(
                                    astB[:],
                                    ag_out[p][half * PT:(half + 1) * PT, NH:N],
                                )
                                nc.vector.scalar_tensor_tensor(
                                    attThalf[kk],
                                    astB[:], s1_bc[:], tmp[:],
                                    op0=mybir.AluOpType.mult,
                                    op1=mybir.AluOpType.add,
                                )

                    # last-pair select, split in two stages so only stage B
                    # (one op per kk) depends on the later-arriving chunk:
                    # attThalf[kk][:, cpair*512:+512] =
                    #     s0*ag_chunk[cpair] + s1*ag_chunk[cpair+2]
                    def sel_stageA(cpair):
                        for ki, kk in enumerate((3, 7)):
                            half = kk // MI
                            a0 = agst.tile([PT, 512], BF16, name="a0", tag="a0")
                            nc.sync.dma_start(
                                a0[:], ag_chunk[cpair][half * PT:(half + 1) * PT, :]
                            )
                            nc.vector.tensor_scalar_mul(
                                tsel[cpair * 2 + ki][:], a0[:], s0_bc[:]
                            )

                    def sel_stageB(cpair):
                        cs = cpair * 512
                        for ki, kk in enumerate((3, 7)):
                            half = kk // MI
                            a2 = agst.tile([PT, 512], BF16, name="a2", tag="a2")
                            eng = nc.scalar if (cpair == 1 and ki == 1) else nc.sync
                            eng.dma_start(
                                a2[:], ag_chunk[cpair + 2][half * PT:(half + 1) * PT, :]
                            )
                            nc.vector.scalar_tensor_tensor(
                                attThalf[kk][:, cs:cs + 512],
                                a2[:], s1_bc[:], tsel[cpair * 2 + ki][:],
                                op0=mybir.AluOpType.mult,
                                op1=mybir.AluOpType.add,
                            )

                    def self_emit_vmm(p, outs, iq, j, at):
                        oA, oB = outs[iq]
                        nc.tensor.matmul(
                            oA[:], vsb[j][:, 2 * p, 0:65], at[:, 0:512],
                            start=(j == 0), stop=(j == MS - 1),
                        )
                        nc.tensor.matmul(
                            oB[:], vsb[j][:, 2 * p + 1, 0:65], at[:, 512:1024],
                            start=(j == 0), stop=(j == MS - 1),
                        )

                    def self_finalize(p, outs, iq, last):
                        # on the very last block the scalar engine is idle and
                        # these copies gate the final AllGather chunk: use it
                        tail = last and iq == 3
                        # both heads' denominators share one tile so the
                        # normalization is a single recip + single broadcast
                        den = finp.tile([1, 1024], F32, name="den", tag="den")
                        for hh, o in enumerate(outs[iq]):
                            seg = attoutT[p][hh * 64:(hh + 1) * 64,
                                             iq * 512:(iq + 1) * 512]
                            dslice = den[:, hh * 512:(hh + 1) * 512]
                            if tail and hh == 0:
                                # split the last block's copies across both
                                # engines: scalar is idle once exps finish
                                nc.scalar.copy(seg, o[0:64, :])
                                nc.scalar.copy(dslice, o[64:65, :])
                            else:
                                nc.vector.tensor_copy(seg, o[0:64, :])
                                nc.vector.tensor_copy(dslice, o[64:65, :])
                        recip = finp.tile([1, 1024], F32, name="recip",
                                          tag="recip")
                        nc.vector.reciprocal_approx_fast(recip[:], den[:])
                        bc = finp.tile([PT, 1024], F32, name="bc", tag="bc")
                        nc.gpsimd.partition_broadcast(bc[:], recip[:])
                        for hh in range(2):
                            seg = attoutT[p][hh * 64:(hh + 1) * 64,
                                             iq * 512:(iq + 1) * 512]
                            nc.vector.tensor_tensor(
                                seg, seg,
                                bc[hh * 64:(hh + 1) * 64,
                                   hh * 512:(hh + 1) * 512],
                                op=mybir.AluOpType.mult,
                            )
                        if last:
                            ag_iq(p, iq)

                    # q/k projections for pair m as half-chains in the hosting
                    # attention's per-step slack (replaces the standalone
                    # qkproj bursts at pair boundaries). k-chains first: kT[m]
                    # is consumed across ALL column chunks from step ~12 of
                    # attention(m), while qT[m] chunk iq is needed only when
                    # its iq block starts.
                    qk_chain = {}

                    def half_chain(m, wsel, ch, half):
                        w_bf, dstT = ((wk_bf, kT), (wq_bf, qT))[wsel]
                        key = (m, wsel, ch)
                        if half == 0:
                            ph = psP.tile([PT, 512], F32, name="ph", tag="psP")
                            qk_chain[key] = ph
                            ks = range(0, KD // 2)
                        else:
                            ph = qk_chain.pop(key)
                            ks = range(KD // 2, KD)
                        for k in ks:
                            nc.tensor.matmul(
                                ph[:],
                                w_bf[k][:, m * PT:(m + 1) * PT],
                                xT[k][:, ch * 512:(ch + 1) * 512],
                                start=(k == 0), stop=(k == KD - 1),
                            )
                        if half == 1:
                            nc.vector.tensor_copy(
                                dstT[m][:, ch * 512:(ch + 1) * 512], ph[:]
                            )

                    def chain_fill(sched, s0, stride, pre=None):
                        # sched: list of (m, wsel, ch) chains, emitted as
                        # half-chains every `stride` steps from step s0
                        def f(iq, j):
                            s = iq * MS + j
                            if pre is not None:
                                pre(iq, j)
                            if s < s0 or (s - s0) % stride != 0:
                                return
                            hc = (s - s0) // stride
                            if hc < 2 * len(sched):
                                c, half = divmod(hc, 2)
                                half_chain(*sched[c], half)
                        return f

                    # global fill balance (per-window TensorE budget is
                    # 64 exp-steps ~= 171k cycles; own scores+vmm = 98k):
                    #   att(0): vproj 6..15 + q0-ch2/3 + k1-all + q1-ch0
                    #   att(1): q1-ch1..3 + all of pair 2 + k3-all + q3-ch0
                    #   att(2): q3-ch1..3 + out-proj pass 1a
                    #   att(3): pass 1b + selects + pass 2a
                    sched0 = [(0, 1, 2), (0, 1, 3),
                              (1, 0, 0), (1, 0, 1), (1, 0, 2), (1, 0, 3),
                              (1, 1, 0)]

                    def pre0(iq, j):
                        if iq == 0:
                            vproj_s(j, psP, "psP")

                    fill0 = chain_fill(sched0, MS, 2, pre=pre0)

                    sched1 = [(1, 1, 1), (1, 1, 2), (1, 1, 3),
                              (2, 0, 0), (2, 1, 0), (2, 0, 1), (2, 0, 2),
                              (2, 0, 3), (2, 1, 1), (2, 1, 2), (2, 1, 3),
                              (3, 0, 0), (3, 0, 1), (3, 0, 2), (3, 0, 3),
                              (3, 1, 0)]
                    fill1 = chain_fill(sched1, 0, 2)

                    sched2 = [(3, 1, 1), (3, 1, 2), (3, 1, 3)]

                    def fill2(iq, j):
                        s = iq * MS + j
                        if s < 6:
                            c, half = divmod(s, 2)
                            half_chain(*sched2[c], half)
                        elif 12 <= s < 44 and (s - 12) % 2 == 0:
                            p1a((s - 12) // 2)

                    # out-projection pass 1a (pairs 0+1 + bias -> stage,
                    # hosted in attention(2)) and 1b (pair 2 added in-place,
                    # hosted in attention(3))
                    def p1a(t):
                        m, nn = divmod(t, 2)
                        pp = psP.tile([PT, 512], F32, name="pp", tag="psP")
                        for ki, kk in enumerate((0, 4, 1, 5)):
                            nc.tensor.matmul(
                                pp[:], attThalf[kk][:, m * PT:(m + 1) * PT],
                                wo_bf[kk][:, nn * 512:(nn + 1) * 512],
                                start=(ki == 0), stop=(ki == 3),
                            )
                        nc.vector.tensor_tensor(
                            stage[t][:], pp[:], bias_st[nn][:],
                            op=mybir.AluOpType.add,
                        )

                    def p1b(t):
                        m, nn = divmod(t, 2)
                        pp = psP.tile([PT, 512], F32, name="pb", tag="psP")
                        for ki, kk in enumerate((2, 6)):
                            nc.tensor.matmul(
                                pp[:], attThalf[kk][:, m * PT:(m + 1) * PT],
                                wo_bf[kk][:, nn * 512:(nn + 1) * 512],
                                start=(ki == 0), stop=(ki == 1),
                            )
                        nc.vector.tensor_tensor(
                            stage[t][:], pp[:], stage[t][:],
                            op=mybir.AluOpType.add,
                        )

                    # pass 2: adds pair-3's contribution (attThalf[3]/[7]) to
                    # the staged pass-1 partials and stores bf16 output rows
                    p2_chains = {}

                    def p2_half(m, nn, half, pool=None, tail=False):
                        t = m * 2 + nn
                        if half == 0:
                            pool = pool if pool is not None else psP
                            pp = pool.tile([PT, 512], F32, name="p2", tag="psP")
                            p2_chains[t] = pp
                            nc.tensor.matmul(
                                pp[:], attThalf[3][:, m * PT:(m + 1) * PT],
                                wo_bf[3][:, nn * 512:(nn + 1) * 512],
                                start=True, stop=False,
                            )
                        else:
                            pp = p2_chains.pop(t)
                            nc.tensor.matmul(
                                pp[:], attThalf[7][:, m * PT:(m + 1) * PT],
                                wo_bf[7][:, nn * 512:(nn + 1) * 512],
                                start=False, stop=True,
                            )
                            osb = osbp.tile([PT, 512], BF16, name="osb", tag="osb")
                            nc.vector.tensor_tensor(
                                osb[:], pp[:], stage[t][:],
                                op=mybir.AluOpType.add,
                            )
                            eng = nc.scalar if tail else nc.sync
                            eng.dma_start(
                                out_ext[m * PT:(m + 1) * PT,
                                        nn * 512:(nn + 1) * 512],
                                osb[:],
                            )

                    # attention(3) i-chunk order (0,2,1,3): chunks 0+2 (the
                    # cpair-0 select inputs) are both gathered by mid-pass, so
                    # the m0..3 output rows complete inside the attention
                    # window; only chunk 3's AG + m4..7 remain at the tail.
                    ORD3 = (0, 2, 1, 3)

                    def fill3(iq, j):
                        si = ORD3.index(iq) * MS + j
                        if 8 <= si < 40 and (si - 8) % 2 == 0:
                            p1b((si - 8) // 2)
                        if si == 20:
                            sel_stageA(0)
                        elif si == 40:
                            sel_stageB(0)
                        elif si == 58:
                            sel_stageA(1)

                    # prologue: pair-0 q/k chains through the psO slots
                    # (scores/exp use psS/psP and start as soon as the
                    # first two chains land; the vmm accumulators naturally
                    # queue behind the remaining chains)
                    for w_bf, dstT, ch in (
                        (wk_bf, kT, 0), (wq_bf, qT, 0), (wk_bf, kT, 1),
                        (wk_bf, kT, 2), (wk_bf, kT, 3), (wq_bf, qT, 1),
                    ):
                        pq = psO.tile([PT, 512], F32, name="pq", tag="psO")
                        for k in range(KD):
                            nc.tensor.matmul(
                                pq[:],
                                w_bf[k][:, 0:PT],
                                xT[k][:, ch * 512:(ch + 1) * 512],
                                start=(k == 0), stop=(k == KD - 1),
                            )
                        nc.vector.tensor_copy(
                            dstT[0][:, ch * 512:(ch + 1) * 512], pq[:]
                        )

                    attention(0, fill=fill0)
                    attention(1, fill=fill1)
                    attention(2, fill=fill2)
                    attention(3, fill=fill3, iq_order=ORD3)

                # ==== tail ====
                # pass-2a (m0..3, select already done mid-window) is emitted
                # first so its matmuls/adds/stores hide under chunk 3's
                # AllGather flight; then the AG-gated cpair-1 select + m4..7
                with tc.tile_pool(name="pso", bufs=4, space="PSUM") as pso_p:
                    for m in range(0, 4):
                        for nn in range(2):
                            p2_half(m, nn, 0, pool=pso_p)
                            p2_half(m, nn, 1)
                    sel_stageB(1)
                    for m in range(4, 8):
                        for nn in range(2):
                            p2_half(m, nn, 0, pool=pso_p)
                            p2_half(m, nn, 1, tail=True)

    nc.compile()
    return nc


def _shard_inputs(x, Wq, Wkv, Wout, bout):
    BF = ml_dtypes.bfloat16
    x = np.asarray(x, dtype=np.float32)
    wq_g = [np.ascontiguousarray(Wq[:, g * IN:(g + 1) * IN]).astype(BF) for g in range(2)]
    wk_g = [np.ascontiguousarray(Wkv[:, g * IN:(g + 1) * IN]).astype(BF) for g in range(2)]
    wv_g = [
        np.ascontiguousarray(Wkv[:, D + g * IN:D + (g + 1) * IN]).astype(BF)
        for g in range(2)
    ]
    wo = np.ascontiguousarray(Wout).astype(BF)
    bo = np.ascontiguousarray(bout, dtype=np.float32)
    xt_b = [np.ascontiguousarray(x[b].T).astype(BF) for b in range(B)]
    in_maps = []
    for c in range(N_CORES):
        b, g = c // 2, c % 2
        sel = np.zeros((1, 2), dtype=np.float32)
        sel[0, g] = 1.0
        in_maps.append({
            "xt": xt_b[b],
            "wq": wq_g[g],
            "wk": wk_g[g],
            "wv": wv_g[g],
            "wout": wo,
            "bout": bo,
            "sel": sel,
        })
    return in_maps


def kernel(x, Wq, Wkv, Wout, bout):
    global _COMPILED
    if _COMPILED is None:
        _COMPILED = build()
    nc = _COMPILED
    in_maps = _shard_inputs(
        np.asarray(x), np.asarray(Wq), np.asarray(Wkv), np.asarray(Wout),
        np.asarray(bout),
    )
    res = bass_utils.run_bass_kernel_spmd(nc, in_maps, core_ids=list(range(N_CORES)))
    out = np.empty((B, N, D), dtype=np.float32)
    for c in range(N_CORES):
        b, g = c // 2, c % 2
        out[b, g * NH:(g + 1) * NH, :] = np.asarray(
            res.results[c]["out"], dtype=np.float32
        )
    return out


if __name__ == "__main__":
    rng = np.random.default_rng(0)
    x = rng.standard_normal((B, N, D)).astype(np.float32)
    Wq = rng.standard_normal((D, D)).astype(np.float32) * D ** -0.5
    Wkv = rng.standard_normal((D, 2 * D)).astype(np.float32) * D ** -0.5
    Wout = rng.standard_normal((D, D)).astype(np.float32) * D ** -0.5
    bout = np.zeros((D,), dtype=np.float32)
    y = kernel(x=x, Wq=Wq, Wkv=Wkv, Wout=Wout, bout=bout)
    print("out shape:", y.shape, "finite:", np.isfinite(y).all())

